# revision 35
# baseline (speedup 1.0000x reference)
"""Tensor-parallel GPT-J-style attention block on 8 TRN2 NeuronCores (v14).

Sharding (vLLM-style TP): w_qkv column-sharded (2 heads/core), attention
per-core over its heads, AllToAll re-shards head-major -> token-major,
w_out contracted per token slice, outputs concatenated (no all-reduce
needed since each core owns its token slice exactly).

HW exec time: ~1.255 ms (from 2.06 ms baseline), NTFF-profiled, core skew
<7us across the 8 cores. PE busy ~1.22 ms ~= the 81.25%-duty-throttled
matmul roofline for the 72.5 GFLOP/core workload; remaining gaps ~40us
(DMA cold-start ~10us, phase boundaries, collective windows).

Key structure (found via perfetto/NTFF trace iteration):
  - QKV: all weight chunks SBUF-resident, token-block-outer loop, rhs
    (hidT) read once; weights stream column-half-first so the startup-
    critical bytes are halved; first matmul issues ~7us in.
  - attention: per-k-tile interleave of scores -> exp -> (denominator
    ones-matmul + PV accumulate); softmax finalize per q-half with the
    tail half deferred into the next instance's instruction stream (PE
    never waits on the recip chain); reciprocal_approx_fast (18-bit) on
    DVE; probsT tiles sized to the causal span.
  - collectives: tiny warm-up AllToAll at kernel start absorbs stream
    setup; per-head AllToAll launched as soon as that head's batches
    finish; gathers of the AllToAll results are emitted after all
    attention and issued from the scalar queue (any queue DMA waiting on
    a collective head-of-line-blocks everything behind it - measured
    22-43us when placed wrong); per-source gather slices so projection
    starts on source 0 immediately.
  - projection: head-0 halves of the first 6 output tiles run during
    AllToAll-1 (psum stashed in bf16); output-tile DMAs go out on the
    scalar queue so sync is dedicated to the weight stream.
"""
import math
import sys

import numpy as np

try:
    import concourse.bass  # noqa: F401
except ImportError:
    sys.path.insert(0, "/opt/trn_rl_repo")

import concourse.mybir as mybir
import concourse.tile as tile
from concourse import bacc
from concourse.bass_utils import run_bass_kernel_spmd
from concourse.masks import make_identity, make_upper_triangular

dt = mybir.dt

N_CORES = 8
B = 4
NH = 16
D = 256
HID = NH * D  # 4096
ROT = D // 2  # 128
RH = ROT // 2  # 64
HPC = NH // N_CORES  # heads per core
QKV_COLS = 3 * HPC * D  # 1536
KT = HID // 128  # 32 contraction tiles
SCALE = 1.0 / math.sqrt(D)
ROPE_BASE = 10000.0

_BUILD_CACHE = {}


def build(S, phases=('qkv', 'attn', 'proj')):
    TOK = B * S
    TS = TOK // N_CORES  # per-core token slice == QKV token-block width
    assert TS == 512 and S % TS == 0
    NTB = N_CORES
    NKT8 = S // 128  # k-token tiles per attention instance
    NQH = max(1, S // 512)  # q halves per attention instance
    QW = min(S, 512)
    f32, f32r, bf16 = dt.float32, dt.float32r, dt.bfloat16
    RG = [list(range(N_CORES))]

    nc = bacc.Bacc("TRN2", target_bir_lowering=False, debug=False,
                   num_devices=N_CORES)

    # ---- I/O (host-retiled layouts; see make_in_maps)
    cos_in = nc.dram_tensor("cos_t", [RH, TOK], f32, kind="ExternalInput")
    sin_in = nc.dram_tensor("sin_t", [RH, TOK], f32, kind="ExternalInput")
    hidT_in = nc.dram_tensor("hidT_t", [128, KT, TOK], bf16, kind="ExternalInput")
    wqkv_in = nc.dram_tensor("w_qkv_t", [128, KT, QKV_COLS], bf16,
                             kind="ExternalInput")
    wout_in = nc.dram_tensor("w_out_t", [128, KT, HID], bf16, kind="ExternalInput")
    out_f = nc.dram_tensor("out_f_0", [TS, HID], f32, kind="ExternalOutput")

    # ---- internal DRAM
    qkvT_d = nc.dram_tensor("qkvT_d", [2 * HPC * D, TOK], bf16)
    vtok_d = [nc.dram_tensor(f"vtok_d{h}", [TOK, D], bf16) for h in range(HPC)]
    a2a_in = [nc.dram_tensor(f"a2a_in{h}", [N_CORES, D, TS], bf16) for h in range(HPC)]
    a2a_out = [nc.dram_tensor(f"a2a_out{h}", [N_CORES, D, TS], bf16)
               for h in range(HPC)]
    warm_in = nc.dram_tensor("warm_in", [N_CORES, 16], bf16)
    warm_out = nc.dram_tensor("warm_out", [N_CORES, 16], bf16)

    WKC = 4                      # kt tiles per weight chunk
    NWC = KT // WKC              # 8 chunks, all resident
    WRC = 8                      # kt tiles per rhs chunk
    NRC = KT // WRC              # 4 chunks per token block
    NM = 6                       # psum column groups per pass

    with tile.TileContext(nc) as tc, \
         tc.tile_pool(name="const", bufs=1) as cpool:
        # allocate long-lived const tiles up front (no emission yet) so the
        # const pool region is settled before the phase pools stack on it
        wtile = cpool.tile([N_CORES, 16], bf16)
        ident = cpool.tile([128, 128], f32)
        ident_b = cpool.tile([128, 128], bf16)
        ones_f = cpool.tile([128, 1], f32)
        ones_b = cpool.tile([128, 1], bf16)
        onesrow_f = cpool.tile([1, 128], f32)
        onesrow_r = cpool.tile([1, 128], f32r)
        tri_f = cpool.tile([128, 128], f32)
        tri_b = cpool.tile([128, 128], bf16)
        with tc.tile_pool(name="qkv_w", bufs=1) as wq_pool, \
             tc.tile_pool(name="qkv_rhs", bufs=2) as rhs_pool, \
             tc.tile_pool(name="qkv_st", bufs=2) as stg_pool, \
             tc.tile_pool(name="qkv_rope", bufs=1) as rope_pool, \
             tc.tile_pool(name="qkv_ps", bufs=1, space="PSUM") as qps_pool:
            cos_sb = rope_pool.tile([RH, TOK], f32)
            sin_sb = rope_pool.tile([RH, TOK], f32)

            # ---- critical-path loads first: the first matmul needs w chunk 0
            # and the first rhs chunk; everything else streams in behind.
            # weights load per column-half: pass 0 only needs cols 0:768,
            # so the startup-critical stream is half the weight bytes.
            HC = QKV_COLS // 2
            w_map = {}

            def load_wchunk(c, h):
                w = wq_pool.tile([128, WKC * HC], bf16, tag=f"w{c}_{h}")
                nc.sync.dma_start(
                    out=w[:],
                    in_=wqkv_in.ap()[:, WKC * c:WKC * (c + 1),
                                     HC * h:HC * (h + 1)])
                w_map[(c, h)] = w

            def load_rhs(tb):
                blks = []
                for r in range(NRC):
                    blk = rhs_pool.tile([128, WRC * TS], bf16, tag=f"rhs{r}",
                                        name=f"rhs{r}_{tb}",
                                        bufs=2 if r < 2 else 1)
                    nc.sync.dma_start(
                        out=blk[:],
                        in_=hidT_in.ap()[:, WRC * r:WRC * (r + 1),
                                         TS * tb:TS * (tb + 1)])
                    blks.append(blk)
                return blks

            # interleave weight and rhs chunk loads so the PE's weight
            # stream keeps pace with its ~5us/chunk consumption rate
            load_wchunk(0, 0)
            blks0 = [None] * NRC

            def load_rhs0(r):
                blk = rhs_pool.tile([128, WRC * TS], bf16, tag=f"rhs{r}",
                                    name=f"rhs{r}_0",
                                    bufs=2 if r < 2 else 1)
                nc.sync.dma_start(
                    out=blk[:], in_=hidT_in.ap()[:, WRC * r:WRC * (r + 1), 0:TS])
                blks0[r] = blk

            load_rhs0(0)
            load_wchunk(1, 0)
            load_wchunk(2, 0)
            load_rhs0(1)
            load_wchunk(3, 0)
            load_wchunk(4, 0)
            load_rhs0(2)
            load_wchunk(5, 0)
            load_wchunk(6, 0)
            load_rhs0(3)
            load_wchunk(7, 0)

            # rope tables (needed by the first staging, ~40us in)
            nc.sync.dma_start(out=cos_sb[:], in_=cos_in.ap())
            nc.sync.dma_start(out=sin_sb[:], in_=sin_in.ap())

            for c in range(NWC):
                load_wchunk(c, 1)

            # warm-up collective: absorbs first-collective stream setup cost
            # while QKV runs; payload is throwaway.
            nc.vector.memset(wtile[:], 0.0)
            nc.sync.dma_start(out=warm_in.ap(), in_=wtile[:])
            nc.gpsimd.collective_compute(
                "AllToAll", mybir.AluOpType.bypass, replica_groups=RG,
                ins=[warm_in.ap().opt()], outs=[warm_out.ap().opt()])

            # constants
            make_identity(nc, ident[:])
            nc.vector.tensor_copy(out=ident_b[:], in_=ident[:])
            nc.vector.memset(ones_f[:], 1.0)
            nc.vector.tensor_copy(out=ones_b[:], in_=ones_f[:])
            nc.vector.memset(onesrow_f[:], 1.0)
            nc.vector.tensor_copy(out=onesrow_r[:], in_=onesrow_f[:])
            make_upper_triangular(nc, tri_f[:], val=1.0, diag=True)
            nc.vector.tensor_copy(out=tri_b[:], in_=tri_f[:])

            # ---- phase 1: QKV projection, token-block outer, both column
            # passes inner sharing one rhs load.
            ROPE_GC = {g * 256 for g in range(2 * HPC)}  # first 128 cols of q/k

            def w_sl(kt, m12):
                w = w_map[(kt // WKC, m12 // NM)]
                base = (kt % WKC) * HC + 128 * (m12 % NM)
                return w[:, base:base + 128]

            qkv_tbs = range(NTB) if 'qkv' in phases else range(0)
            for tb in qkv_tbs:
                blks = blks0 if tb == 0 else load_rhs(tb)

                def rhs_sl(kt):
                    base = (kt % WRC) * TS
                    return blks[kt // WRC][:, base:base + TS]

                for p in range(2):
                    ps = [qps_pool.tile([128, TS], dt.float32, tag=f"qkvps{m}",
                                        name=f"qkvps{m}_{p}_{tb}")
                          for m in range(NM)]
                    for c in range(NWC):
                        for m in range(NM):
                            for i in range(WKC):
                                kt = WKC * c + i
                                nc.tensor.matmul(
                                    out=ps[m][:],
                                    lhsT=w_sl(kt, NM * p + m),
                                    rhs=rhs_sl(kt),
                                    start=(kt == 0), stop=(kt == KT - 1))
                    # staging: q/k columns -> merged dst_all spill (+rope);
                    # v columns -> transpose to token-major, merged spill
                    nqk = 6 if p == 0 else 2
                    if nqk:
                        dst_all = stg_pool.tile([128, nqk * TS], bf16,
                                                tag="qst", name=f"qst_{p}_{tb}")
                    for m in range(NM):
                        gc = NM * 128 * p + 128 * m
                        if gc < 2 * HPC * D:
                            dsl = dst_all[:, TS * m:TS * (m + 1)]
                            if gc in ROPE_GC:
                                c_ = cos_sb[:, TS * tb:TS * (tb + 1)]
                                s_ = sin_sb[:, TS * tb:TS * (tb + 1)]
                                t1 = stg_pool.tile([RH, TS], f32, tag="rt1", bufs=1)
                                t2 = stg_pool.tile([RH, TS], f32, tag="rt2", bufs=1)
                                nc.vector.tensor_mul(t1[:], ps[m][0:RH, :], c_)
                                nc.vector.tensor_mul(t2[:], ps[m][RH:2 * RH, :], s_)
                                nc.vector.tensor_sub(dsl[0:RH, :], t1[:], t2[:])
                                t3 = stg_pool.tile([RH, TS], f32, tag="rt3", bufs=1)
                                t4 = stg_pool.tile([RH, TS], f32, tag="rt4", bufs=1)
                                nc.vector.tensor_mul(t3[:], ps[m][RH:2 * RH, :], c_)
                                nc.vector.tensor_mul(t4[:], ps[m][0:RH, :], s_)
                                nc.vector.tensor_add(dsl[RH:2 * RH, :], t3[:], t4[:])
                            else:
                                nc.vector.tensor_copy(out=dsl, in_=ps[m][:])
                        else:
                            hl_ = (gc - 2 * HPC * D) // D
                            d0 = (gc - 2 * HPC * D) % D
                            dstv = stg_pool.tile([128, TS], bf16, tag="dstv",
                                                 name=f"dstv_{p}_{tb}_{m}")
                            nc.vector.tensor_copy(out=dstv[:], in_=ps[m][:])
                            tpv = qps_pool.tile([128, TS], bf16, tag="vtp",
                                                name=f"vtp_{p}_{tb}_{m}", bufs=2)
                            for q8 in range(TS // 128):
                                nc.tensor.transpose(
                                    tpv[:, 128 * q8:128 * (q8 + 1)],
                                    dstv[:, 128 * q8:128 * (q8 + 1)],
                                    ident_b[:])
                            vst = stg_pool.tile([128, TS], bf16, tag="vst",
                                                name=f"vst_{p}_{tb}_{m}")
                            nc.vector.tensor_copy(out=vst[:], in_=tpv[:])
                            tok0 = TS * tb
                            nc.sync.dma_start(
                                out=vtok_d[hl_].ap()[tok0:tok0 + TS, d0:d0 + 128]
                                .rearrange("(q p) d -> p q d", p=128),
                                in_=vst[:])
                    if nqk:
                        gc0 = NM * 128 * p
                        nc.sync.dma_start(
                            out=qkvT_d.ap()[gc0:gc0 + nqk * 128,
                                            TS * tb:TS * (tb + 1)]
                            .rearrange("(g p) t -> p g t", p=128),
                            in_=dst_all[:])

        # ---- phase 2+3: attention, AllToAll, output projection
        attn_on = 'attn' in phases
        NMT = (TS + 127) // 128
        NNT = HID // 512 if 'proj' in phases else 0
        kts = [(hl, src, sub) for hl in range(HPC)
               for src in range(N_CORES) for sub in range(2)]
        with tc.tile_pool(name="att_in", bufs=6) as ain_pool, \
             tc.tile_pool(name="att_vt", bufs=3) as avt_pool, \
             tc.tile_pool(name="att_pr", bufs=2) as apr_pool, \
             tc.tile_pool(name="att_o", bufs=2) as aout_pool, \
             tc.tile_pool(name="op_a", bufs=1) as oa_pool, \
             tc.tile_pool(name="op_w", bufs=3) as ow_pool, \
             tc.tile_pool(name="op_f", bufs=3) as of_pool, \
             tc.tile_pool(name="att_sc", bufs=3, space="PSUM") as scps_pool, \
             tc.tile_pool(name="att_av", bufs=1, space="PSUM") as avps_pool:
            am_all = {}

            def launch_a2a(hl):
                nc.gpsimd.collective_compute(
                    "AllToAll", mybir.AluOpType.bypass, replica_groups=RG,
                    ins=[a2a_in[hl].ap().opt()], outs=[a2a_out[hl].ap().opt()])

            def load_amt(hl):
                # emitted only after all attention (a queue DMA waiting on
                # the collective would head-of-line-block work behind it);
                # per-source slices so the projection can start on source 0
                # while the rest of the gather is still in flight
                amt = oa_pool.tile([128, 2 * N_CORES * TS], bf16,
                                   tag=f"am{hl}")
                with tc.tile_wait_until(1.0 + 0.06 * hl):
                    for s in range(N_CORES):
                        eng = nc.scalar
                        eng.dma_start(
                            out=amt[:, 2 * TS * s:2 * TS * (s + 1)],
                            in_=a2a_out[hl].ap()[s:s + 1]
                            .rearrange("s (u p) t -> p s u t", p=128))
                am_all[hl] = amt

            # last k-tile contributing to each q half
            stops = [min(NKT8 - 1, (QW * (qh + 1) - 1) // 128)
                     for qh in range(NQH)]

            def emit_instance(hl, b, prev_final, post_prev):
                tok0 = S * b
                qoff = D * hl
                koff = HPC * D + D * hl

                def load_pair(off, nm):
                    ts_ = []
                    for dtile in range(2):
                        t = ain_pool.tile([128, S], bf16, tag=f"{nm}{dtile}",
                                          name=f"{nm}{dtile}_{hl}_{b}")
                        nc.sync.dma_start(
                            out=t[:],
                            in_=qkvT_d.ap()[off + 128 * dtile:
                                            off + 128 * (dtile + 1),
                                            tok0:tok0 + S])
                        ts_.append(t)
                    return ts_

                qT = load_pair(qoff, "q")
                kT = load_pair(koff, "k")
                vt_all = avt_pool.tile([128, NKT8 * D], bf16, tag="vtok",
                                       name=f"vtok_{hl}_{b}")
                nc.sync.dma_start(
                    out=vt_all[:],
                    in_=vtok_d[hl].ap()[tok0:tok0 + S, :]
                    .rearrange("(k p) d -> p k d", p=128))

                def v_sl(kt8, dtile):
                    base = kt8 * D
                    return vt_all[:, base + 128 * dtile:base + 128 * (dtile + 1)]

                # each probsT tile holds only the causal span q >= 128*k
                probsT = [apr_pool.tile([128, S - 128 * k], bf16, tag=f"pr{k}",
                                        name=f"pr{k}_{hl}_{b}")
                          for k in range(NKT8)]
                sm = avps_pool.tile([1 + 32 * (NQH - 1), QW], dt.float32,
                                    tag="sm", name=f"sm_{hl}_{b}")
                av = [[avps_pool.tile([128, QW], dt.float32, tag=f"av{d}{q}",
                                      name=f"av{d}{q}_{hl}_{b}")
                       for q in range(NQH)] for d in range(2)]
                att_sb = [aout_pool.tile([128, S], bf16, tag=f"attn{d}",
                                         name=f"attn{d}_{hl}_{b}")
                          for d in range(2)]
                sums_sb = aout_pool.tile([1, S], f32, tag="sums",
                                         name=f"sums_{hl}_{b}")
                recip = aout_pool.tile([1, S], f32, tag="recip",
                                       name=f"recip_{hl}_{b}")
                recip_r = aout_pool.tile([1, S], f32r, tag="recipr",
                                         name=f"recipr_{hl}_{b}")
                rbc = aout_pool.tile([128, S], f32, tag="rbc",
                                     name=f"rbc_{hl}_{b}", bufs=1)

                def qk(kt8):
                    pr = probsT[kt8]
                    qlo = 128 * kt8
                    q0 = qlo
                    while q0 < S:
                        wch = min(512, S - q0)
                        pss = scps_pool.tile([128, QW], dt.float32, tag="scps",
                                             name=f"sc_{hl}_{b}_{kt8}_{q0}")
                        for dtile in range(2):
                            nc.tensor.matmul(
                                out=pss[:, 0:wch],
                                lhsT=kT[dtile][:, 128 * kt8:128 * (kt8 + 1)],
                                rhs=qT[dtile][:, q0:q0 + wch],
                                start=(dtile == 0), stop=(dtile == 1))
                        nc.scalar.activation(
                            out=pr[:, q0 - qlo:q0 + wch - qlo], in_=pss[:, 0:wch],
                            func=mybir.ActivationFunctionType.Exp, scale=SCALE)
                        q0 += wch
                    nc.vector.tensor_mul(pr[:, 0:128], pr[:, 0:128], tri_b[:])

                def spv(kt8):
                    pr = probsT[kt8]
                    qlo = 128 * kt8
                    for qh in range(NQH):
                        q0, q1 = QW * qh, QW * (qh + 1)
                        lo = max(qlo, q0)
                        if lo >= q1:
                            continue
                        st, sp = (kt8 == 0), (kt8 == stops[qh])
                        nc.tensor.matmul(
                            out=sm[32 * qh:32 * qh + 1, lo - q0:q1 - q0],
                            lhsT=ones_b[:], rhs=pr[:, lo - qlo:q1 - qlo],
                            start=st, stop=sp)
                        for dtl in range(2):
                            nc.tensor.matmul(
                                out=av[dtl][qh][:, lo - q0:q1 - q0],
                                lhsT=v_sl(kt8, dtl), rhs=pr[:, lo - qlo:q1 - qlo],
                                start=st, stop=sp)

                def dve_chain(qh):
                    q0, q1 = QW * qh, QW * (qh + 1)
                    nc.scalar.copy(out=sums_sb[:, q0:q1],
                                   in_=sm[32 * qh:32 * qh + 1, :])
                    nc.vector.reciprocal_approx_fast(
                        out=recip[:, q0:q1], in_=sums_sb[:, q0:q1])
                    nc.vector.tensor_copy(out=recip_r[:, q0:q1],
                                          in_=recip[:, q0:q1])

                def bcast_norm(qh):
                    q0, q1 = QW * qh, QW * (qh + 1)
                    bcp = scps_pool.tile([128, QW], dt.float32, tag="scps",
                                         name=f"bcp_{hl}_{b}_{qh}")
                    nc.tensor.matmul(out=bcp[:], lhsT=onesrow_r[:],
                                     rhs=recip_r[:, q0:q1],
                                     start=True, stop=True)
                    nc.vector.tensor_copy(out=rbc[:, q0:q1], in_=bcp[:])
                    for dtl in range(2):
                        nc.vector.tensor_mul(att_sb[dtl][:, q0:q1],
                                             av[dtl][qh][:], rbc[:, q0:q1])

                def scatter():
                    dest0 = (S * b) // TS
                    nu = S // TS
                    for dtl in range(2):
                        nc.sync.dma_start(
                            out=a2a_in[hl].ap()[dest0:dest0 + nu,
                                                128 * dtl:128 * (dtl + 1), :]
                            .rearrange("u p t -> p u t"),
                            in_=att_sb[dtl][:])

                # staggered emission: scores for k-tile k+2 run while k's
                # probs finish on scalar/vector; softmax finalize for the
                # first q-half lands mid-instance, the second is deferred
                # into the next instance's stream.
                qk(0)
                if NKT8 > 1:
                    qk(1)
                if prev_final:
                    prev_final()
                if post_prev:
                    post_prev()
                for k in range(NKT8):
                    if k + 2 < NKT8:
                        qk(k + 2)
                    spv(k)
                    if k == stops[0]:
                        dve_chain(0)
                    if NQH > 1 and k == stops[0] + 1:
                        bcast_norm(0)
                if NQH == 1:
                    bcast_norm(0)
                    def final():
                        scatter()
                else:
                    dve_chain(1)

                    def final():
                        bcast_norm(1)
                        scatter()
                return final

            prev_final = None
            post_prev = None
            if attn_on:
                for hl in range(HPC):
                    for b in range(B):
                        prev_final = emit_instance(hl, b, prev_final, post_prev)
                        post_prev = None
                    post_prev = (lambda hl=hl: launch_a2a(hl))
                prev_final()
                post_prev()
                if 'proj' in phases:
                    for hl in range(HPC):
                        load_amt(hl)

            def am_sl(ki, mt, mm):
                hl, src, sub = kts[ki]
                base = (src * 2 + sub) * TS
                return am_all[hl][:, base + 128 * mt:base + 128 * mt + mm]

            PKB = 8  # k-tiles per w block
            NKB = (len(kts) + PKB - 1) // PKB
            NSP = 6 if NNT else 0  # nt's whose head-0 part runs early
            stash = {}

            def proj_part(nt, kbs):
                ps_f = [avps_pool.tile([min(128, TS), 512], dt.float32,
                                       tag=f"av{mt // 2}{mt % 2}",
                                       name=f"f{mt}_{nt}_{kbs[0]}")
                        for mt in range(NMT)]
                for kb in kbs:
                    kis = list(range(PKB * kb, min(PKB * (kb + 1), len(kts))))
                    wblk = ow_pool.tile([128, PKB * 512], bf16, tag="wr",
                                        name=f"wr_{nt}_{kb}")
                    nc.sync.dma_start(
                        out=wblk[:],
                        in_=wout_in.ap()[:, PKB * kb:PKB * (kb + 1),
                                         512 * nt:512 * (nt + 1)])
                    for mt in range(NMT):
                        mm = min(128, TS - 128 * mt)
                        for i, ki in enumerate(kis):
                            nc.tensor.matmul(
                                out=ps_f[mt][:],
                                lhsT=am_sl(ki, mt, mm),
                                rhs=wblk[:, 512 * i:512 * (i + 1)],
                                start=(ki == PKB * kbs[0]),
                                stop=(ki == PKB * (kbs[-1] + 1) - 1))
                return ps_f

            # head-0 half for the first NSP nt's (overlaps a2a of head 1)
            for nt in range(NSP):
                ps_f = proj_part(nt, list(range(NKB // 2)))
                for mt in range(NMT):
                    st = oa_pool.tile([min(128, TS), 512], bf16,
                                      tag=f"sth{nt}{mt}", name=f"sth{nt}{mt}")
                    nc.vector.tensor_copy(out=st[:], in_=ps_f[mt][:])
                    stash[(nt, mt)] = st
            for nt in range(NNT):
                if nt < NSP:
                    ps_f = proj_part(nt, list(range(NKB // 2, NKB)))
                else:
                    ps_f = proj_part(nt, list(range(NKB)))
                for mt in range(NMT):
                    mm = min(128, TS - 128 * mt)
                    fo = of_pool.tile([min(128, TS), 512], f32, tag="fo")
                    if nt < NSP:
                        nc.vector.tensor_add(fo[:], ps_f[mt][:],
                                             stash[(nt, mt)][:])
                    else:
                        nc.scalar.copy(out=fo[:], in_=ps_f[mt][:])
                    nc.scalar.dma_start(
                        out=out_f.ap()[128 * mt:128 * mt + mm,
                                       512 * nt:512 * (nt + 1)],
                        in_=fo[:])

    nc.compile()
    return nc


def get_nc(S, phases=('qkv', 'attn', 'proj')):
    key = (S, tuple(phases))
    if key not in _BUILD_CACHE:
        _BUILD_CACHE[key] = build(S, phases=phases)
    return _BUILD_CACHE[key]


def make_in_maps(position_ids, hidden_states, w_qkv, w_out):
    import ml_dtypes
    S = hidden_states.shape[1]
    TOK = B * S
    flat = np.asarray(hidden_states, dtype=np.float32).reshape(TOK, HID)
    hidT = flat.T.astype(ml_dtypes.bfloat16)          # [HID, TOK]
    hidT_t = np.ascontiguousarray(
        hidT.reshape(KT, 128, TOK).transpose(1, 0, 2))  # [128, KT, TOK]
    pos = np.asarray(position_ids).reshape(TOK).astype(np.float32)
    invf = (1.0 / (ROPE_BASE ** (np.arange(0, ROT, 2, dtype=np.float32) / ROT)))
    ang = invf[:, None] * pos[None, :]  # [RH, TOK]
    cos_t = np.cos(ang).astype(np.float32)
    sin_t = np.sin(ang).astype(np.float32)
    w_qkv = np.asarray(w_qkv, dtype=np.float32)
    w_out = np.asarray(w_out, dtype=np.float32)
    kts = [(hl, src, sub) for hl in range(HPC)
           for src in range(N_CORES) for sub in range(2)]
    wout_rows = np.stack([
        w_out[512 * src + 256 * hl + 128 * sub:
              512 * src + 256 * hl + 128 * sub + 128]
        for (hl, src, sub) in kts])                    # [32, 128, HID]
    wout_t = np.ascontiguousarray(
        wout_rows.transpose(1, 0, 2)).astype(ml_dtypes.bfloat16)  # [128, 32, HID]
    in_maps = []
    for c in range(N_CORES):
        c0 = HPC * D * c
        wq = np.concatenate([w_qkv[:, c0:c0 + HPC * D],
                             w_qkv[:, HID + c0:HID + c0 + HPC * D],
                             w_qkv[:, 2 * HID + c0:2 * HID + c0 + HPC * D]],
                            axis=1)                    # [HID, QKV_COLS]
        wq_t = np.ascontiguousarray(
            wq.reshape(KT, 128, QKV_COLS).transpose(1, 0, 2)
        ).astype(ml_dtypes.bfloat16)                   # [128, KT, QKV_COLS]
        in_maps.append({
            "cos_t": cos_t,
            "sin_t": sin_t,
            "hidT_t": hidT_t,
            "w_qkv_t": wq_t,
            "w_out_t": wout_t,
        })
    return in_maps


def kernel(position_ids, hidden_states, w_qkv, w_out):
    S = hidden_states.shape[1]
    nc = get_nc(S)
    in_maps = make_in_maps(position_ids, hidden_states, w_qkv, w_out)
    res = run_bass_kernel_spmd(nc, in_maps, list(range(N_CORES)))
    TOK = B * S
    out = np.concatenate([res.results[c]["out_f_0"] for c in range(N_CORES)], axis=0)
    return out.reshape(B, S, HID).astype(np.float32)


# revision 36
# speedup vs baseline: 1.0002x; 1.0002x over previous
"""Tensor-parallel GPT-J-style attention block on 8 TRN2 NeuronCores (v14).

Sharding (vLLM-style TP): w_qkv column-sharded (2 heads/core), attention
per-core over its heads, AllToAll re-shards head-major -> token-major,
w_out contracted per token slice, outputs concatenated (no all-reduce
needed since each core owns its token slice exactly).

HW exec time: ~1.255 ms (from 2.06 ms baseline), NTFF-profiled, core skew
<7us across the 8 cores. PE busy ~1.22 ms ~= the 81.25%-duty-throttled
matmul roofline for the 72.5 GFLOP/core workload; remaining gaps ~40us
(DMA cold-start ~10us, phase boundaries, collective windows).

Key structure (found via perfetto/NTFF trace iteration):
  - QKV: all weight chunks SBUF-resident, token-block-outer loop, rhs
    (hidT) read once; weights stream column-half-first so the startup-
    critical bytes are halved; first matmul issues ~7us in.
  - attention: per-k-tile interleave of scores -> exp -> (denominator
    ones-matmul + PV accumulate); softmax finalize per q-half with the
    tail half deferred into the next instance's instruction stream (PE
    never waits on the recip chain); reciprocal_approx_fast (18-bit) on
    DVE; probsT tiles sized to the causal span.
  - collectives: tiny warm-up AllToAll at kernel start absorbs stream
    setup; per-head AllToAll launched as soon as that head's batches
    finish; gathers of the AllToAll results are emitted after all
    attention and issued from the scalar queue (any queue DMA waiting on
    a collective head-of-line-blocks everything behind it - measured
    22-43us when placed wrong); per-source gather slices so projection
    starts on source 0 immediately.
  - projection: head-0 halves of the first 6 output tiles run during
    AllToAll-1 (psum stashed in bf16); output-tile DMAs go out on the
    scalar queue so sync is dedicated to the weight stream.
"""
import math
import sys

import numpy as np

try:
    import concourse.bass  # noqa: F401
except ImportError:
    sys.path.insert(0, "/opt/trn_rl_repo")

import concourse.mybir as mybir
import concourse.tile as tile
from concourse import bacc
from concourse.bass_utils import run_bass_kernel_spmd
from concourse.masks import make_identity, make_upper_triangular

dt = mybir.dt

N_CORES = 8
B = 4
NH = 16
D = 256
HID = NH * D  # 4096
ROT = D // 2  # 128
RH = ROT // 2  # 64
HPC = NH // N_CORES  # heads per core
QKV_COLS = 3 * HPC * D  # 1536
KT = HID // 128  # 32 contraction tiles
SCALE = 1.0 / math.sqrt(D)
ROPE_BASE = 10000.0

_BUILD_CACHE = {}


def build(S, phases=('qkv', 'attn', 'proj')):
    TOK = B * S
    TS = TOK // N_CORES  # per-core token slice == QKV token-block width
    assert TS == 512 and S % TS == 0
    NTB = N_CORES
    NKT8 = S // 128  # k-token tiles per attention instance
    NQH = max(1, S // 512)  # q halves per attention instance
    QW = min(S, 512)
    f32, f32r, bf16 = dt.float32, dt.float32r, dt.bfloat16
    RG = [list(range(N_CORES))]

    nc = bacc.Bacc("TRN2", target_bir_lowering=False, debug=False,
                   num_devices=N_CORES)

    # ---- I/O (host-retiled layouts; see make_in_maps)
    cos_in = nc.dram_tensor("cos_t", [RH, TOK], f32, kind="ExternalInput")
    sin_in = nc.dram_tensor("sin_t", [RH, TOK], f32, kind="ExternalInput")
    hidT_in = nc.dram_tensor("hidT_t", [128, KT, TOK], bf16, kind="ExternalInput")
    wqkv_in = nc.dram_tensor("w_qkv_t", [128, KT, QKV_COLS], bf16,
                             kind="ExternalInput")
    wout_in = nc.dram_tensor("w_out_t", [128, KT, HID], bf16, kind="ExternalInput")
    out_f = nc.dram_tensor("out_f_0", [TS, HID], f32, kind="ExternalOutput")

    # ---- internal DRAM
    qkvT_d = nc.dram_tensor("qkvT_d", [2 * HPC * D, TOK], bf16)
    vtok_d = [nc.dram_tensor(f"vtok_d{h}", [TOK, D], bf16) for h in range(HPC)]
    a2a_in = [nc.dram_tensor(f"a2a_in{h}", [N_CORES, D, TS], bf16) for h in range(HPC)]
    a2a_out = [nc.dram_tensor(f"a2a_out{h}", [N_CORES, D, TS], bf16)
               for h in range(HPC)]
    warm_in = nc.dram_tensor("warm_in", [N_CORES, 16], bf16)
    warm_out = nc.dram_tensor("warm_out", [N_CORES, 16], bf16)

    WKC = 4                      # kt tiles per weight chunk
    NWC = KT // WKC              # 8 chunks, all resident
    WRC = 8                      # kt tiles per rhs chunk
    NRC = KT // WRC              # 4 chunks per token block
    NM = 6                       # psum column groups per pass

    with tile.TileContext(nc) as tc, \
         tc.tile_pool(name="const", bufs=1) as cpool:
        # allocate long-lived const tiles up front (no emission yet) so the
        # const pool region is settled before the phase pools stack on it
        wtile = cpool.tile([N_CORES, 16], bf16)
        ident = cpool.tile([128, 128], f32)
        ident_b = cpool.tile([128, 128], bf16)
        ones_f = cpool.tile([128, 1], f32)
        ones_b = cpool.tile([128, 1], bf16)
        onesrow_f = cpool.tile([1, 128], f32)
        onesrow_r = cpool.tile([1, 128], f32r)
        tri_f = cpool.tile([128, 128], f32)
        tri_b = cpool.tile([128, 128], bf16)
        with tc.tile_pool(name="qkv_w", bufs=1) as wq_pool, \
             tc.tile_pool(name="qkv_rhs", bufs=2) as rhs_pool, \
             tc.tile_pool(name="qkv_st", bufs=2) as stg_pool, \
             tc.tile_pool(name="qkv_rope", bufs=1) as rope_pool, \
             tc.tile_pool(name="qkv_ps", bufs=1, space="PSUM") as qps_pool:
            cos_sb = rope_pool.tile([RH, TOK], f32)
            sin_sb = rope_pool.tile([RH, TOK], f32)

            # ---- critical-path loads first: the first matmul needs w chunk 0
            # and the first rhs chunk; everything else streams in behind.
            # weights load per column-half: pass 0 only needs cols 0:768,
            # so the startup-critical stream is half the weight bytes.
            HC = QKV_COLS // 2
            w_map = {}

            def load_wchunk(c, h):
                w = wq_pool.tile([128, WKC * HC], bf16, tag=f"w{c}_{h}")
                nc.sync.dma_start(
                    out=w[:],
                    in_=wqkv_in.ap()[:, WKC * c:WKC * (c + 1),
                                     HC * h:HC * (h + 1)])
                w_map[(c, h)] = w

            def load_rhs(tb):
                blks = []
                for r in range(NRC):
                    blk = rhs_pool.tile([128, WRC * TS], bf16, tag=f"rhs{r}",
                                        name=f"rhs{r}_{tb}",
                                        bufs=2 if r < 2 else 1)
                    nc.sync.dma_start(
                        out=blk[:],
                        in_=hidT_in.ap()[:, WRC * r:WRC * (r + 1),
                                         TS * tb:TS * (tb + 1)])
                    blks.append(blk)
                return blks

            # interleave weight and rhs chunk loads so the PE's weight
            # stream keeps pace with its ~5us/chunk consumption rate
            load_wchunk(0, 0)
            blks0 = [None] * NRC

            def load_rhs0(r):
                blk = rhs_pool.tile([128, WRC * TS], bf16, tag=f"rhs{r}",
                                    name=f"rhs{r}_0",
                                    bufs=2 if r < 2 else 1)
                nc.sync.dma_start(
                    out=blk[:], in_=hidT_in.ap()[:, WRC * r:WRC * (r + 1), 0:TS])
                blks0[r] = blk

            load_rhs0(0)
            load_wchunk(1, 0)
            load_wchunk(2, 0)
            load_rhs0(1)
            load_wchunk(3, 0)
            load_wchunk(4, 0)
            load_rhs0(2)
            load_wchunk(5, 0)
            load_wchunk(6, 0)
            load_rhs0(3)
            load_wchunk(7, 0)

            # rope tables (needed by the first staging, ~40us in)
            nc.sync.dma_start(out=cos_sb[:], in_=cos_in.ap())
            nc.sync.dma_start(out=sin_sb[:], in_=sin_in.ap())

            for c in range(NWC):
                load_wchunk(c, 1)

            # warm-up collective: absorbs first-collective stream setup cost
            # while QKV runs; payload is throwaway.
            nc.vector.memset(wtile[:], 0.0)
            nc.sync.dma_start(out=warm_in.ap(), in_=wtile[:])
            nc.gpsimd.collective_compute(
                "AllToAll", mybir.AluOpType.bypass, replica_groups=RG,
                ins=[warm_in.ap().opt()], outs=[warm_out.ap().opt()])

            # constants
            make_identity(nc, ident[:])
            nc.vector.tensor_copy(out=ident_b[:], in_=ident[:])
            nc.vector.memset(ones_f[:], 1.0)
            nc.vector.tensor_copy(out=ones_b[:], in_=ones_f[:])
            nc.vector.memset(onesrow_f[:], 1.0)
            nc.vector.tensor_copy(out=onesrow_r[:], in_=onesrow_f[:])
            make_upper_triangular(nc, tri_f[:], val=1.0, diag=True)
            nc.vector.tensor_copy(out=tri_b[:], in_=tri_f[:])

            # ---- phase 1: QKV projection, token-block outer, both column
            # passes inner sharing one rhs load.
            ROPE_GC = {g * 256 for g in range(2 * HPC)}  # first 128 cols of q/k

            def w_sl(kt, m12):
                w = w_map[(kt // WKC, m12 // NM)]
                base = (kt % WKC) * HC + 128 * (m12 % NM)
                return w[:, base:base + 128]

            qkv_tbs = range(NTB) if 'qkv' in phases else range(0)
            for tb in qkv_tbs:
                blks = blks0 if tb == 0 else load_rhs(tb)

                def rhs_sl(kt):
                    base = (kt % WRC) * TS
                    return blks[kt // WRC][:, base:base + TS]

                for p in range(2):
                    ps = [qps_pool.tile([128, TS], dt.float32, tag=f"qkvps{m}",
                                        name=f"qkvps{m}_{p}_{tb}")
                          for m in range(NM)]
                    for c in range(NWC):
                        for m in range(NM):
                            for i in range(WKC):
                                kt = WKC * c + i
                                nc.tensor.matmul(
                                    out=ps[m][:],
                                    lhsT=w_sl(kt, NM * p + m),
                                    rhs=rhs_sl(kt),
                                    start=(kt == 0), stop=(kt == KT - 1))
                    # staging: q/k columns -> merged dst_all spill (+rope);
                    # v columns -> transpose to token-major, merged spill
                    nqk = 6 if p == 0 else 2
                    if nqk:
                        dst_all = stg_pool.tile([128, nqk * TS], bf16,
                                                tag="qst", name=f"qst_{p}_{tb}")
                    for m in range(NM):
                        gc = NM * 128 * p + 128 * m
                        if gc < 2 * HPC * D:
                            dsl = dst_all[:, TS * m:TS * (m + 1)]
                            if gc in ROPE_GC:
                                c_ = cos_sb[:, TS * tb:TS * (tb + 1)]
                                s_ = sin_sb[:, TS * tb:TS * (tb + 1)]
                                t1 = stg_pool.tile([RH, TS], f32, tag="rt1", bufs=1)
                                t2 = stg_pool.tile([RH, TS], f32, tag="rt2", bufs=1)
                                nc.vector.tensor_mul(t1[:], ps[m][0:RH, :], c_)
                                nc.vector.tensor_mul(t2[:], ps[m][RH:2 * RH, :], s_)
                                nc.vector.tensor_sub(dsl[0:RH, :], t1[:], t2[:])
                                t3 = stg_pool.tile([RH, TS], f32, tag="rt3", bufs=1)
                                t4 = stg_pool.tile([RH, TS], f32, tag="rt4", bufs=1)
                                nc.vector.tensor_mul(t3[:], ps[m][RH:2 * RH, :], c_)
                                nc.vector.tensor_mul(t4[:], ps[m][0:RH, :], s_)
                                nc.vector.tensor_add(dsl[RH:2 * RH, :], t3[:], t4[:])
                            else:
                                nc.vector.tensor_copy(out=dsl, in_=ps[m][:])
                        else:
                            hl_ = (gc - 2 * HPC * D) // D
                            d0 = (gc - 2 * HPC * D) % D
                            dstv = stg_pool.tile([128, TS], bf16, tag="dstv",
                                                 name=f"dstv_{p}_{tb}_{m}")
                            nc.vector.tensor_copy(out=dstv[:], in_=ps[m][:])
                            tpv = qps_pool.tile([128, TS], bf16, tag="vtp",
                                                name=f"vtp_{p}_{tb}_{m}", bufs=2)
                            for q8 in range(TS // 128):
                                nc.tensor.transpose(
                                    tpv[:, 128 * q8:128 * (q8 + 1)],
                                    dstv[:, 128 * q8:128 * (q8 + 1)],
                                    ident_b[:])
                            vst = stg_pool.tile([128, TS], bf16, tag="vst",
                                                name=f"vst_{p}_{tb}_{m}")
                            nc.vector.tensor_copy(out=vst[:], in_=tpv[:])
                            tok0 = TS * tb
                            nc.sync.dma_start(
                                out=vtok_d[hl_].ap()[tok0:tok0 + TS, d0:d0 + 128]
                                .rearrange("(q p) d -> p q d", p=128),
                                in_=vst[:])
                    if nqk:
                        gc0 = NM * 128 * p
                        nc.sync.dma_start(
                            out=qkvT_d.ap()[gc0:gc0 + nqk * 128,
                                            TS * tb:TS * (tb + 1)]
                            .rearrange("(g p) t -> p g t", p=128),
                            in_=dst_all[:])

        # ---- phase 2+3: attention, AllToAll, output projection
        attn_on = 'attn' in phases
        NMT = (TS + 127) // 128
        NNT = HID // 512 if 'proj' in phases else 0
        kts = [(hl, src, sub) for hl in range(HPC)
               for src in range(N_CORES) for sub in range(2)]
        with tc.tile_pool(name="att_in", bufs=6) as ain_pool, \
             tc.tile_pool(name="att_vt", bufs=3) as avt_pool, \
             tc.tile_pool(name="att_pr", bufs=2) as apr_pool, \
             tc.tile_pool(name="att_o", bufs=2) as aout_pool, \
             tc.tile_pool(name="op_a", bufs=1) as oa_pool, \
             tc.tile_pool(name="op_w", bufs=3) as ow_pool, \
             tc.tile_pool(name="op_f", bufs=3) as of_pool, \
             tc.tile_pool(name="att_sc", bufs=3, space="PSUM") as scps_pool, \
             tc.tile_pool(name="att_av", bufs=1, space="PSUM") as avps_pool:
            am_all = {}

            def launch_a2a(hl):
                nc.gpsimd.collective_compute(
                    "AllToAll", mybir.AluOpType.bypass, replica_groups=RG,
                    ins=[a2a_in[hl].ap().opt()], outs=[a2a_out[hl].ap().opt()])

            def load_amt(hl):
                # emitted only after all attention (a queue DMA waiting on
                # the collective would head-of-line-block work behind it);
                # per-source slices so the projection can start on source 0
                # while the rest of the gather is still in flight
                amt = oa_pool.tile([128, 2 * N_CORES * TS], bf16,
                                   tag=f"am{hl}")
                with tc.tile_wait_until(1.0 + 0.06 * hl):
                    for s in range(N_CORES):
                        eng = nc.scalar
                        eng.dma_start(
                            out=amt[:, 2 * TS * s:2 * TS * (s + 1)],
                            in_=a2a_out[hl].ap()[s:s + 1]
                            .rearrange("s (u p) t -> p s u t", p=128))
                am_all[hl] = amt

            # last k-tile contributing to each q half
            stops = [min(NKT8 - 1, (QW * (qh + 1) - 1) // 128)
                     for qh in range(NQH)]

            def emit_instance(hl, b, prev_final, post_prev):
                tok0 = S * b
                qoff = D * hl
                koff = HPC * D + D * hl

                def load_pair(off, nm):
                    ts_ = []
                    for dtile in range(2):
                        t = ain_pool.tile([128, S], bf16, tag=f"{nm}{dtile}",
                                          name=f"{nm}{dtile}_{hl}_{b}")
                        nc.sync.dma_start(
                            out=t[:],
                            in_=qkvT_d.ap()[off + 128 * dtile:
                                            off + 128 * (dtile + 1),
                                            tok0:tok0 + S])
                        ts_.append(t)
                    return ts_

                qT = load_pair(qoff, "q")
                kT = load_pair(koff, "k")
                vt_all = avt_pool.tile([128, NKT8 * D], bf16, tag="vtok",
                                       name=f"vtok_{hl}_{b}")
                nc.sync.dma_start(
                    out=vt_all[:],
                    in_=vtok_d[hl].ap()[tok0:tok0 + S, :]
                    .rearrange("(k p) d -> p k d", p=128))

                def v_sl(kt8, dtile):
                    base = kt8 * D
                    return vt_all[:, base + 128 * dtile:base + 128 * (dtile + 1)]

                # each probsT tile holds only the causal span q >= 128*k
                probsT = [apr_pool.tile([128, S - 128 * k], bf16, tag=f"pr{k}",
                                        name=f"pr{k}_{hl}_{b}")
                          for k in range(NKT8)]
                sm = avps_pool.tile([1 + 32 * (NQH - 1), QW], dt.float32,
                                    tag="sm", name=f"sm_{hl}_{b}")
                av = [[avps_pool.tile([128, QW], dt.float32, tag=f"av{d}{q}",
                                      name=f"av{d}{q}_{hl}_{b}")
                       for q in range(NQH)] for d in range(2)]
                att_sb = [aout_pool.tile([128, S], bf16, tag=f"attn{d}",
                                         name=f"attn{d}_{hl}_{b}")
                          for d in range(2)]
                sums_sb = aout_pool.tile([1, S], f32, tag="sums",
                                         name=f"sums_{hl}_{b}")
                recip = aout_pool.tile([1, S], f32, tag="recip",
                                       name=f"recip_{hl}_{b}")
                recip_r = aout_pool.tile([1, S], f32r, tag="recipr",
                                         name=f"recipr_{hl}_{b}")
                rbc = aout_pool.tile([128, S], f32, tag="rbc",
                                     name=f"rbc_{hl}_{b}", bufs=1)

                def qk(kt8):
                    pr = probsT[kt8]
                    qlo = 128 * kt8
                    q0 = qlo
                    while q0 < S:
                        wch = min(512, S - q0)
                        pss = scps_pool.tile([128, QW], dt.float32, tag="scps",
                                             name=f"sc_{hl}_{b}_{kt8}_{q0}")
                        for dtile in range(2):
                            nc.tensor.matmul(
                                out=pss[:, 0:wch],
                                lhsT=kT[dtile][:, 128 * kt8:128 * (kt8 + 1)],
                                rhs=qT[dtile][:, q0:q0 + wch],
                                start=(dtile == 0), stop=(dtile == 1))
                        nc.scalar.activation(
                            out=pr[:, q0 - qlo:q0 + wch - qlo], in_=pss[:, 0:wch],
                            func=mybir.ActivationFunctionType.Exp, scale=SCALE)
                        q0 += wch
                    nc.vector.tensor_mul(pr[:, 0:128], pr[:, 0:128], tri_b[:])

                def spv(kt8):
                    pr = probsT[kt8]
                    qlo = 128 * kt8
                    for qh in range(NQH):
                        q0, q1 = QW * qh, QW * (qh + 1)
                        lo = max(qlo, q0)
                        if lo >= q1:
                            continue
                        st, sp = (kt8 == 0), (kt8 == stops[qh])
                        nc.tensor.matmul(
                            out=sm[32 * qh:32 * qh + 1, lo - q0:q1 - q0],
                            lhsT=ones_b[:], rhs=pr[:, lo - qlo:q1 - qlo],
                            start=st, stop=sp)
                        for dtl in range(2):
                            nc.tensor.matmul(
                                out=av[dtl][qh][:, lo - q0:q1 - q0],
                                lhsT=v_sl(kt8, dtl), rhs=pr[:, lo - qlo:q1 - qlo],
                                start=st, stop=sp)

                def dve_chain(qh):
                    q0, q1 = QW * qh, QW * (qh + 1)
                    nc.scalar.copy(out=sums_sb[:, q0:q1],
                                   in_=sm[32 * qh:32 * qh + 1, :])
                    nc.vector.reciprocal_approx_fast(
                        out=recip[:, q0:q1], in_=sums_sb[:, q0:q1])
                    nc.vector.tensor_copy(out=recip_r[:, q0:q1],
                                          in_=recip[:, q0:q1])

                def bcast_norm(qh):
                    q0, q1 = QW * qh, QW * (qh + 1)
                    bcp = scps_pool.tile([128, QW], dt.float32, tag="scps",
                                         name=f"bcp_{hl}_{b}_{qh}")
                    nc.tensor.matmul(out=bcp[:], lhsT=onesrow_r[:],
                                     rhs=recip_r[:, q0:q1],
                                     start=True, stop=True)
                    nc.vector.tensor_copy(out=rbc[:, q0:q1], in_=bcp[:])
                    for dtl in range(2):
                        nc.vector.tensor_mul(att_sb[dtl][:, q0:q1],
                                             av[dtl][qh][:], rbc[:, q0:q1])

                def scatter():
                    dest0 = (S * b) // TS
                    nu = S // TS
                    for dtl in range(2):
                        nc.sync.dma_start(
                            out=a2a_in[hl].ap()[dest0:dest0 + nu,
                                                128 * dtl:128 * (dtl + 1), :]
                            .rearrange("u p t -> p u t"),
                            in_=att_sb[dtl][:])

                # staggered emission: scores for k-tile k+2 run while k's
                # probs finish on scalar/vector; softmax finalize for the
                # first q-half lands mid-instance, the second is deferred
                # into the next instance's stream.
                qk(0)
                if NKT8 > 1:
                    qk(1)
                if prev_final:
                    prev_final()
                if post_prev:
                    post_prev()
                for k in range(NKT8):
                    if k + 2 < NKT8:
                        qk(k + 2)
                    spv(k)
                    if k == stops[0]:
                        dve_chain(0)
                    if NQH > 1 and k == stops[0] + 1:
                        bcast_norm(0)
                if NQH == 1:
                    bcast_norm(0)
                    def final():
                        scatter()
                else:
                    dve_chain(1)

                    def final():
                        bcast_norm(1)
                        scatter()
                return final

            prev_final = None
            post_prev = None
            if attn_on:
                for hl in range(HPC):
                    for b in range(B):
                        prev_final = emit_instance(hl, b, prev_final, post_prev)
                        post_prev = None
                    post_prev = (lambda hl=hl: launch_a2a(hl))
                prev_final()
                post_prev()
                if 'proj' in phases:
                    for hl in range(HPC):
                        load_amt(hl)

            def am_sl(ki, mt, mm):
                hl, src, sub = kts[ki]
                base = (src * 2 + sub) * TS
                return am_all[hl][:, base + 128 * mt:base + 128 * mt + mm]

            PKB = 8  # k-tiles per w block
            NKB = (len(kts) + PKB - 1) // PKB
            NSP = 6 if NNT else 0  # nt's whose head-0 part runs early
            stash = {}

            def proj_part(nt, kbs):
                ps_f = [avps_pool.tile([min(128, TS), 512], dt.float32,
                                       tag=f"av{mt // 2}{mt % 2}",
                                       name=f"f{mt}_{nt}_{kbs[0]}")
                        for mt in range(NMT)]
                for kb in kbs:
                    kis = list(range(PKB * kb, min(PKB * (kb + 1), len(kts))))
                    wblk = ow_pool.tile([128, PKB * 512], bf16, tag="wr",
                                        name=f"wr_{nt}_{kb}")
                    # the first few weight blocks have no dependencies:
                    # stage them mid-QKV so the projection start never
                    # waits on its weight stream
                    early = nt == 0 or (nt == 1 and kb == kbs[0])
                    ctx = tc.high_priority() if early else None
                    if ctx:
                        ctx.__enter__()
                    with tc.tile_wait_until(0.5, enable=early):
                        nc.sync.dma_start(
                            out=wblk[:],
                            in_=wout_in.ap()[:, PKB * kb:PKB * (kb + 1),
                                             512 * nt:512 * (nt + 1)])
                    if ctx:
                        ctx.__exit__(None, None, None)
                    for mt in range(NMT):
                        mm = min(128, TS - 128 * mt)
                        for i, ki in enumerate(kis):
                            nc.tensor.matmul(
                                out=ps_f[mt][:],
                                lhsT=am_sl(ki, mt, mm),
                                rhs=wblk[:, 512 * i:512 * (i + 1)],
                                start=(ki == PKB * kbs[0]),
                                stop=(ki == PKB * (kbs[-1] + 1) - 1))
                return ps_f

            # head-0 half for the first NSP nt's (overlaps a2a of head 1)
            for nt in range(NSP):
                ps_f = proj_part(nt, list(range(NKB // 2)))
                for mt in range(NMT):
                    st = oa_pool.tile([min(128, TS), 512], bf16,
                                      tag=f"sth{nt}{mt}", name=f"sth{nt}{mt}")
                    nc.vector.tensor_copy(out=st[:], in_=ps_f[mt][:])
                    stash[(nt, mt)] = st
            for nt in range(NNT):
                if nt < NSP:
                    ps_f = proj_part(nt, list(range(NKB // 2, NKB)))
                else:
                    ps_f = proj_part(nt, list(range(NKB)))
                for mt in range(NMT):
                    mm = min(128, TS - 128 * mt)
                    fo = of_pool.tile([min(128, TS), 512], f32, tag="fo")
                    if nt < NSP:
                        nc.vector.tensor_add(fo[:], ps_f[mt][:],
                                             stash[(nt, mt)][:])
                    else:
                        nc.scalar.copy(out=fo[:], in_=ps_f[mt][:])
                    nc.scalar.dma_start(
                        out=out_f.ap()[128 * mt:128 * mt + mm,
                                       512 * nt:512 * (nt + 1)],
                        in_=fo[:])

    nc.compile()
    return nc


def get_nc(S, phases=('qkv', 'attn', 'proj')):
    key = (S, tuple(phases))
    if key not in _BUILD_CACHE:
        _BUILD_CACHE[key] = build(S, phases=phases)
    return _BUILD_CACHE[key]


def make_in_maps(position_ids, hidden_states, w_qkv, w_out):
    import ml_dtypes
    S = hidden_states.shape[1]
    TOK = B * S
    flat = np.asarray(hidden_states, dtype=np.float32).reshape(TOK, HID)
    hidT = flat.T.astype(ml_dtypes.bfloat16)          # [HID, TOK]
    hidT_t = np.ascontiguousarray(
        hidT.reshape(KT, 128, TOK).transpose(1, 0, 2))  # [128, KT, TOK]
    pos = np.asarray(position_ids).reshape(TOK).astype(np.float32)
    invf = (1.0 / (ROPE_BASE ** (np.arange(0, ROT, 2, dtype=np.float32) / ROT)))
    ang = invf[:, None] * pos[None, :]  # [RH, TOK]
    cos_t = np.cos(ang).astype(np.float32)
    sin_t = np.sin(ang).astype(np.float32)
    w_qkv = np.asarray(w_qkv, dtype=np.float32)
    w_out = np.asarray(w_out, dtype=np.float32)
    kts = [(hl, src, sub) for hl in range(HPC)
           for src in range(N_CORES) for sub in range(2)]
    wout_rows = np.stack([
        w_out[512 * src + 256 * hl + 128 * sub:
              512 * src + 256 * hl + 128 * sub + 128]
        for (hl, src, sub) in kts])                    # [32, 128, HID]
    wout_t = np.ascontiguousarray(
        wout_rows.transpose(1, 0, 2)).astype(ml_dtypes.bfloat16)  # [128, 32, HID]
    in_maps = []
    for c in range(N_CORES):
        c0 = HPC * D * c
        wq = np.concatenate([w_qkv[:, c0:c0 + HPC * D],
                             w_qkv[:, HID + c0:HID + c0 + HPC * D],
                             w_qkv[:, 2 * HID + c0:2 * HID + c0 + HPC * D]],
                            axis=1)                    # [HID, QKV_COLS]
        wq_t = np.ascontiguousarray(
            wq.reshape(KT, 128, QKV_COLS).transpose(1, 0, 2)
        ).astype(ml_dtypes.bfloat16)                   # [128, KT, QKV_COLS]
        in_maps.append({
            "cos_t": cos_t,
            "sin_t": sin_t,
            "hidT_t": hidT_t,
            "w_qkv_t": wq_t,
            "w_out_t": wout_t,
        })
    return in_maps


def kernel(position_ids, hidden_states, w_qkv, w_out):
    S = hidden_states.shape[1]
    nc = get_nc(S)
    in_maps = make_in_maps(position_ids, hidden_states, w_qkv, w_out)
    res = run_bass_kernel_spmd(nc, in_maps, list(range(N_CORES)))
    TOK = B * S
    out = np.concatenate([res.results[c]["out_f_0"] for c in range(N_CORES)], axis=0)
    return out.reshape(B, S, HID).astype(np.float32)


# revision 37
# speedup vs baseline: 1.0036x; 1.0034x over previous
"""Tensor-parallel GPT-J-style attention block on 8 TRN2 NeuronCores (v14).

Sharding (vLLM-style TP): w_qkv column-sharded (2 heads/core), attention
per-core over its heads, AllToAll re-shards head-major -> token-major,
w_out contracted per token slice, outputs concatenated (no all-reduce
needed since each core owns its token slice exactly).

HW exec time: ~1.255 ms (from 2.06 ms baseline), NTFF-profiled, core skew
<7us across the 8 cores. PE busy ~1.22 ms ~= the 81.25%-duty-throttled
matmul roofline for the 72.5 GFLOP/core workload; remaining gaps ~40us
(DMA cold-start ~10us, phase boundaries, collective windows).

Key structure (found via perfetto/NTFF trace iteration):
  - QKV: all weight chunks SBUF-resident, token-block-outer loop, rhs
    (hidT) read once; weights stream column-half-first so the startup-
    critical bytes are halved; first matmul issues ~7us in.
  - attention: per-k-tile interleave of scores -> exp -> (denominator
    ones-matmul + PV accumulate); softmax finalize per q-half with the
    tail half deferred into the next instance's instruction stream (PE
    never waits on the recip chain); reciprocal_approx_fast (18-bit) on
    DVE; probsT tiles sized to the causal span.
  - collectives: tiny warm-up AllToAll at kernel start absorbs stream
    setup; per-head AllToAll launched as soon as that head's batches
    finish; gathers of the AllToAll results are emitted after all
    attention and issued from the scalar queue (any queue DMA waiting on
    a collective head-of-line-blocks everything behind it - measured
    22-43us when placed wrong); per-source gather slices so projection
    starts on source 0 immediately.
  - projection: head-0 halves of the first 6 output tiles run during
    AllToAll-1 (psum stashed in bf16); output-tile DMAs go out on the
    scalar queue so sync is dedicated to the weight stream.
"""
import math
import sys

import numpy as np

try:
    import concourse.bass  # noqa: F401
except ImportError:
    sys.path.insert(0, "/opt/trn_rl_repo")

import concourse.mybir as mybir
import concourse.tile as tile
from concourse import bacc
from concourse.bass_utils import run_bass_kernel_spmd
from concourse.masks import make_identity, make_upper_triangular

dt = mybir.dt

N_CORES = 8
B = 4
NH = 16
D = 256
HID = NH * D  # 4096
ROT = D // 2  # 128
RH = ROT // 2  # 64
HPC = NH // N_CORES  # heads per core
QKV_COLS = 3 * HPC * D  # 1536
KT = HID // 128  # 32 contraction tiles
SCALE = 1.0 / math.sqrt(D)
ROPE_BASE = 10000.0

_BUILD_CACHE = {}


def build(S, phases=('qkv', 'attn', 'proj')):
    TOK = B * S
    TS = TOK // N_CORES  # per-core token slice == QKV token-block width
    assert TS == 512 and S % TS == 0
    NTB = N_CORES
    NKT8 = S // 128  # k-token tiles per attention instance
    NQH = max(1, S // 512)  # q halves per attention instance
    QW = min(S, 512)
    f32, f32r, bf16 = dt.float32, dt.float32r, dt.bfloat16
    RG = [list(range(N_CORES))]

    nc = bacc.Bacc("TRN2", target_bir_lowering=False, debug=False,
                   num_devices=N_CORES)

    # ---- I/O (host-retiled layouts; see make_in_maps)
    cos_in = nc.dram_tensor("cos_t", [RH, TOK], f32, kind="ExternalInput")
    sin_in = nc.dram_tensor("sin_t", [RH, TOK], f32, kind="ExternalInput")
    hidT_in = nc.dram_tensor("hidT_t", [128, KT, TOK], bf16, kind="ExternalInput")
    wqkv_in = nc.dram_tensor("w_qkv_t", [128, KT, QKV_COLS], bf16,
                             kind="ExternalInput")
    wout_in = nc.dram_tensor("w_out_t", [128, KT, HID], bf16, kind="ExternalInput")
    out_f = nc.dram_tensor("out_f_0", [TS, HID], f32, kind="ExternalOutput")

    # ---- internal DRAM
    qkvT_d = nc.dram_tensor("qkvT_d", [2 * HPC * D, TOK], bf16)
    vtok_d = [nc.dram_tensor(f"vtok_d{h}", [TOK, D], bf16) for h in range(HPC)]
    a2a_in = [nc.dram_tensor(f"a2a_in{h}", [N_CORES, D, TS], bf16) for h in range(HPC)]
    a2a_out = [nc.dram_tensor(f"a2a_out{h}", [N_CORES, D, TS], bf16)
               for h in range(HPC)]
    warm_in = nc.dram_tensor("warm_in", [N_CORES, 16], bf16)
    warm_out = nc.dram_tensor("warm_out", [N_CORES, 16], bf16)

    WKC = 4                      # kt tiles per weight chunk
    NWC = KT // WKC              # 8 chunks, all resident
    WRC = 8                      # kt tiles per rhs chunk
    NRC = KT // WRC              # 4 chunks per token block
    NM = 6                       # psum column groups per pass

    with tile.TileContext(nc) as tc, \
         tc.tile_pool(name="const", bufs=1) as cpool:
        # allocate long-lived const tiles up front (no emission yet) so the
        # const pool region is settled before the phase pools stack on it
        wtile = cpool.tile([N_CORES, 16], bf16)
        ident = cpool.tile([128, 128], f32)
        ident_b = cpool.tile([128, 128], bf16)
        ones_f = cpool.tile([128, 1], f32)
        ones_b = cpool.tile([128, 1], bf16)
        onesrow_f = cpool.tile([1, 128], f32)
        onesrow_r = cpool.tile([1, 128], f32r)
        tri_f = cpool.tile([128, 128], f32)
        tri_b = cpool.tile([128, 128], bf16)
        with tc.tile_pool(name="qkv_w", bufs=1) as wq_pool, \
             tc.tile_pool(name="qkv_rhs", bufs=2) as rhs_pool, \
             tc.tile_pool(name="qkv_st", bufs=2) as stg_pool, \
             tc.tile_pool(name="qkv_rope", bufs=1) as rope_pool, \
             tc.tile_pool(name="qkv_ps", bufs=1, space="PSUM") as qps_pool:
            cos_sb = rope_pool.tile([RH, TOK], f32)
            sin_sb = rope_pool.tile([RH, TOK], f32)

            # ---- critical-path loads first: the first matmul needs w chunk 0
            # and the first rhs chunk; everything else streams in behind.
            # weights load per column-half: pass 0 only needs cols 0:768,
            # so the startup-critical stream is half the weight bytes.
            HC = QKV_COLS // 2
            w_map = {}

            def load_wchunk(c, h):
                w = wq_pool.tile([128, WKC * HC], bf16, tag=f"w{c}_{h}")
                nc.sync.dma_start(
                    out=w[:],
                    in_=wqkv_in.ap()[:, WKC * c:WKC * (c + 1),
                                     HC * h:HC * (h + 1)])
                w_map[(c, h)] = w

            def load_rhs(tb):
                blks = []
                for r in range(NRC):
                    blk = rhs_pool.tile([128, WRC * TS], bf16, tag=f"rhs{r}",
                                        name=f"rhs{r}_{tb}",
                                        bufs=2 if r < 2 else 1)
                    nc.sync.dma_start(
                        out=blk[:],
                        in_=hidT_in.ap()[:, WRC * r:WRC * (r + 1),
                                         TS * tb:TS * (tb + 1)])
                    blks.append(blk)
                return blks

            # interleave weight and rhs chunk loads so the PE's weight
            # stream keeps pace with its ~5us/chunk consumption rate
            load_wchunk(0, 0)
            blks0 = [None] * NRC

            def load_rhs0(r):
                blk = rhs_pool.tile([128, WRC * TS], bf16, tag=f"rhs{r}",
                                    name=f"rhs{r}_0",
                                    bufs=2 if r < 2 else 1)
                nc.sync.dma_start(
                    out=blk[:], in_=hidT_in.ap()[:, WRC * r:WRC * (r + 1), 0:TS])
                blks0[r] = blk

            load_rhs0(0)
            load_wchunk(1, 0)
            load_wchunk(2, 0)
            load_rhs0(1)
            load_wchunk(3, 0)
            load_wchunk(4, 0)
            load_rhs0(2)
            load_wchunk(5, 0)
            load_wchunk(6, 0)
            load_rhs0(3)
            load_wchunk(7, 0)

            # rope tables (needed by the first staging, ~40us in)
            nc.sync.dma_start(out=cos_sb[:], in_=cos_in.ap())
            nc.sync.dma_start(out=sin_sb[:], in_=sin_in.ap())

            # second column-half (needed from tb0 pass 1, ~50us) interleaved
            # with tb1's rhs (needed ~90us) so neither stream starves
            for c in range(4):
                load_wchunk(c, 1)
            blks1 = load_rhs(1)
            for c in range(4, NWC):
                load_wchunk(c, 1)

            # warm-up collective: absorbs first-collective stream setup cost
            # while QKV runs; payload is throwaway.
            nc.vector.memset(wtile[:], 0.0)
            nc.sync.dma_start(out=warm_in.ap(), in_=wtile[:])
            nc.gpsimd.collective_compute(
                "AllToAll", mybir.AluOpType.bypass, replica_groups=RG,
                ins=[warm_in.ap().opt()], outs=[warm_out.ap().opt()])

            # constants
            make_identity(nc, ident[:])
            nc.vector.tensor_copy(out=ident_b[:], in_=ident[:])
            nc.vector.memset(ones_f[:], 1.0)
            nc.vector.tensor_copy(out=ones_b[:], in_=ones_f[:])
            nc.vector.memset(onesrow_f[:], 1.0)
            nc.vector.tensor_copy(out=onesrow_r[:], in_=onesrow_f[:])
            make_upper_triangular(nc, tri_f[:], val=1.0, diag=True)
            nc.vector.tensor_copy(out=tri_b[:], in_=tri_f[:])

            # ---- phase 1: QKV projection, token-block outer, both column
            # passes inner sharing one rhs load.
            ROPE_GC = {g * 256 for g in range(2 * HPC)}  # first 128 cols of q/k

            def w_sl(kt, m12):
                w = w_map[(kt // WKC, m12 // NM)]
                base = (kt % WKC) * HC + 128 * (m12 % NM)
                return w[:, base:base + 128]

            qkv_tbs = range(NTB) if 'qkv' in phases else range(0)
            for tb in qkv_tbs:
                if tb == 0:
                    blks = blks0
                elif tb == 1:
                    blks = blks1
                else:
                    blks = load_rhs(tb)

                def rhs_sl(kt):
                    base = (kt % WRC) * TS
                    return blks[kt // WRC][:, base:base + TS]

                for p in range(2):
                    ps = [qps_pool.tile([128, TS], dt.float32, tag=f"qkvps{m}",
                                        name=f"qkvps{m}_{p}_{tb}")
                          for m in range(NM)]
                    for c in range(NWC):
                        for m in range(NM):
                            for i in range(WKC):
                                kt = WKC * c + i
                                nc.tensor.matmul(
                                    out=ps[m][:],
                                    lhsT=w_sl(kt, NM * p + m),
                                    rhs=rhs_sl(kt),
                                    start=(kt == 0), stop=(kt == KT - 1))
                    # staging: q/k columns -> merged dst_all spill (+rope);
                    # v columns -> transpose to token-major, merged spill
                    nqk = 6 if p == 0 else 2
                    if nqk:
                        dst_all = stg_pool.tile([128, nqk * TS], bf16,
                                                tag="qst", name=f"qst_{p}_{tb}")
                    for m in range(NM):
                        gc = NM * 128 * p + 128 * m
                        if gc < 2 * HPC * D:
                            dsl = dst_all[:, TS * m:TS * (m + 1)]
                            if gc in ROPE_GC:
                                c_ = cos_sb[:, TS * tb:TS * (tb + 1)]
                                s_ = sin_sb[:, TS * tb:TS * (tb + 1)]
                                t1 = stg_pool.tile([RH, TS], f32, tag="rt1", bufs=1)
                                t2 = stg_pool.tile([RH, TS], f32, tag="rt2", bufs=1)
                                nc.vector.tensor_mul(t1[:], ps[m][0:RH, :], c_)
                                nc.vector.tensor_mul(t2[:], ps[m][RH:2 * RH, :], s_)
                                nc.vector.tensor_sub(dsl[0:RH, :], t1[:], t2[:])
                                t3 = stg_pool.tile([RH, TS], f32, tag="rt3", bufs=1)
                                t4 = stg_pool.tile([RH, TS], f32, tag="rt4", bufs=1)
                                nc.vector.tensor_mul(t3[:], ps[m][RH:2 * RH, :], c_)
                                nc.vector.tensor_mul(t4[:], ps[m][0:RH, :], s_)
                                nc.vector.tensor_add(dsl[RH:2 * RH, :], t3[:], t4[:])
                            else:
                                nc.vector.tensor_copy(out=dsl, in_=ps[m][:])
                        else:
                            hl_ = (gc - 2 * HPC * D) // D
                            d0 = (gc - 2 * HPC * D) % D
                            dstv = stg_pool.tile([128, TS], bf16, tag="dstv",
                                                 name=f"dstv_{p}_{tb}_{m}")
                            nc.vector.tensor_copy(out=dstv[:], in_=ps[m][:])
                            tpv = qps_pool.tile([128, TS], bf16, tag="vtp",
                                                name=f"vtp_{p}_{tb}_{m}", bufs=2)
                            for q8 in range(TS // 128):
                                nc.tensor.transpose(
                                    tpv[:, 128 * q8:128 * (q8 + 1)],
                                    dstv[:, 128 * q8:128 * (q8 + 1)],
                                    ident_b[:])
                            vst = stg_pool.tile([128, TS], bf16, tag="vst",
                                                name=f"vst_{p}_{tb}_{m}")
                            nc.vector.tensor_copy(out=vst[:], in_=tpv[:])
                            tok0 = TS * tb
                            nc.sync.dma_start(
                                out=vtok_d[hl_].ap()[tok0:tok0 + TS, d0:d0 + 128]
                                .rearrange("(q p) d -> p q d", p=128),
                                in_=vst[:])
                    if nqk:
                        gc0 = NM * 128 * p
                        nc.sync.dma_start(
                            out=qkvT_d.ap()[gc0:gc0 + nqk * 128,
                                            TS * tb:TS * (tb + 1)]
                            .rearrange("(g p) t -> p g t", p=128),
                            in_=dst_all[:])

        # ---- phase 2+3: attention, AllToAll, output projection
        attn_on = 'attn' in phases
        NMT = (TS + 127) // 128
        NNT = HID // 512 if 'proj' in phases else 0
        kts = [(hl, src, sub) for hl in range(HPC)
               for src in range(N_CORES) for sub in range(2)]
        with tc.tile_pool(name="att_in", bufs=6) as ain_pool, \
             tc.tile_pool(name="att_vt", bufs=3) as avt_pool, \
             tc.tile_pool(name="att_pr", bufs=2) as apr_pool, \
             tc.tile_pool(name="att_o", bufs=2) as aout_pool, \
             tc.tile_pool(name="op_a", bufs=1) as oa_pool, \
             tc.tile_pool(name="op_w", bufs=3) as ow_pool, \
             tc.tile_pool(name="op_f", bufs=3) as of_pool, \
             tc.tile_pool(name="att_sc", bufs=3, space="PSUM") as scps_pool, \
             tc.tile_pool(name="att_av", bufs=1, space="PSUM") as avps_pool:
            am_all = {}

            def launch_a2a(hl):
                nc.gpsimd.collective_compute(
                    "AllToAll", mybir.AluOpType.bypass, replica_groups=RG,
                    ins=[a2a_in[hl].ap().opt()], outs=[a2a_out[hl].ap().opt()])

            def load_amt(hl):
                # emitted only after all attention (a queue DMA waiting on
                # the collective would head-of-line-block work behind it);
                # per-source slices so the projection can start on source 0
                # while the rest of the gather is still in flight
                amt = oa_pool.tile([128, 2 * N_CORES * TS], bf16,
                                   tag=f"am{hl}")
                with tc.tile_wait_until(1.0 + 0.06 * hl):
                    for s in range(N_CORES):
                        eng = nc.scalar
                        eng.dma_start(
                            out=amt[:, 2 * TS * s:2 * TS * (s + 1)],
                            in_=a2a_out[hl].ap()[s:s + 1]
                            .rearrange("s (u p) t -> p s u t", p=128))
                am_all[hl] = amt

            # last k-tile contributing to each q half
            stops = [min(NKT8 - 1, (QW * (qh + 1) - 1) // 128)
                     for qh in range(NQH)]

            def emit_instance(hl, b, prev_final, post_prev):
                tok0 = S * b
                qoff = D * hl
                koff = HPC * D + D * hl

                def load_pair(off, nm):
                    ts_ = []
                    for dtile in range(2):
                        t = ain_pool.tile([128, S], bf16, tag=f"{nm}{dtile}",
                                          name=f"{nm}{dtile}_{hl}_{b}")
                        nc.sync.dma_start(
                            out=t[:],
                            in_=qkvT_d.ap()[off + 128 * dtile:
                                            off + 128 * (dtile + 1),
                                            tok0:tok0 + S])
                        ts_.append(t)
                    return ts_

                qT = load_pair(qoff, "q")
                kT = load_pair(koff, "k")
                vt_all = avt_pool.tile([128, NKT8 * D], bf16, tag="vtok",
                                       name=f"vtok_{hl}_{b}")
                nc.sync.dma_start(
                    out=vt_all[:],
                    in_=vtok_d[hl].ap()[tok0:tok0 + S, :]
                    .rearrange("(k p) d -> p k d", p=128))

                def v_sl(kt8, dtile):
                    base = kt8 * D
                    return vt_all[:, base + 128 * dtile:base + 128 * (dtile + 1)]

                # each probsT tile holds only the causal span q >= 128*k
                probsT = [apr_pool.tile([128, S - 128 * k], bf16, tag=f"pr{k}",
                                        name=f"pr{k}_{hl}_{b}")
                          for k in range(NKT8)]
                sm = avps_pool.tile([1 + 32 * (NQH - 1), QW], dt.float32,
                                    tag="sm", name=f"sm_{hl}_{b}")
                av = [[avps_pool.tile([128, QW], dt.float32, tag=f"av{d}{q}",
                                      name=f"av{d}{q}_{hl}_{b}")
                       for q in range(NQH)] for d in range(2)]
                att_sb = [aout_pool.tile([128, S], bf16, tag=f"attn{d}",
                                         name=f"attn{d}_{hl}_{b}")
                          for d in range(2)]
                sums_sb = aout_pool.tile([1, S], f32, tag="sums",
                                         name=f"sums_{hl}_{b}")
                recip = aout_pool.tile([1, S], f32, tag="recip",
                                       name=f"recip_{hl}_{b}")
                recip_r = aout_pool.tile([1, S], f32r, tag="recipr",
                                         name=f"recipr_{hl}_{b}")
                rbc = aout_pool.tile([128, S], f32, tag="rbc",
                                     name=f"rbc_{hl}_{b}", bufs=1)

                def qk(kt8):
                    pr = probsT[kt8]
                    qlo = 128 * kt8
                    q0 = qlo
                    while q0 < S:
                        wch = min(512, S - q0)
                        pss = scps_pool.tile([128, QW], dt.float32, tag="scps",
                                             name=f"sc_{hl}_{b}_{kt8}_{q0}")
                        for dtile in range(2):
                            nc.tensor.matmul(
                                out=pss[:, 0:wch],
                                lhsT=kT[dtile][:, 128 * kt8:128 * (kt8 + 1)],
                                rhs=qT[dtile][:, q0:q0 + wch],
                                start=(dtile == 0), stop=(dtile == 1))
                        nc.scalar.activation(
                            out=pr[:, q0 - qlo:q0 + wch - qlo], in_=pss[:, 0:wch],
                            func=mybir.ActivationFunctionType.Exp, scale=SCALE)
                        q0 += wch
                    nc.vector.tensor_mul(pr[:, 0:128], pr[:, 0:128], tri_b[:])

                def spv(kt8):
                    pr = probsT[kt8]
                    qlo = 128 * kt8
                    for qh in range(NQH):
                        q0, q1 = QW * qh, QW * (qh + 1)
                        lo = max(qlo, q0)
                        if lo >= q1:
                            continue
                        st, sp = (kt8 == 0), (kt8 == stops[qh])
                        nc.tensor.matmul(
                            out=sm[32 * qh:32 * qh + 1, lo - q0:q1 - q0],
                            lhsT=ones_b[:], rhs=pr[:, lo - qlo:q1 - qlo],
                            start=st, stop=sp)
                        for dtl in range(2):
                            nc.tensor.matmul(
                                out=av[dtl][qh][:, lo - q0:q1 - q0],
                                lhsT=v_sl(kt8, dtl), rhs=pr[:, lo - qlo:q1 - qlo],
                                start=st, stop=sp)

                def dve_chain(qh):
                    q0, q1 = QW * qh, QW * (qh + 1)
                    nc.scalar.copy(out=sums_sb[:, q0:q1],
                                   in_=sm[32 * qh:32 * qh + 1, :])
                    nc.vector.reciprocal_approx_fast(
                        out=recip[:, q0:q1], in_=sums_sb[:, q0:q1])
                    nc.vector.tensor_copy(out=recip_r[:, q0:q1],
                                          in_=recip[:, q0:q1])

                def bcast_norm(qh):
                    q0, q1 = QW * qh, QW * (qh + 1)
                    bcp = scps_pool.tile([128, QW], dt.float32, tag="scps",
                                         name=f"bcp_{hl}_{b}_{qh}")
                    nc.tensor.matmul(out=bcp[:], lhsT=onesrow_r[:],
                                     rhs=recip_r[:, q0:q1],
                                     start=True, stop=True)
                    nc.vector.tensor_copy(out=rbc[:, q0:q1], in_=bcp[:])
                    for dtl in range(2):
                        nc.vector.tensor_mul(att_sb[dtl][:, q0:q1],
                                             av[dtl][qh][:], rbc[:, q0:q1])

                def scatter():
                    dest0 = (S * b) // TS
                    nu = S // TS
                    for dtl in range(2):
                        nc.sync.dma_start(
                            out=a2a_in[hl].ap()[dest0:dest0 + nu,
                                                128 * dtl:128 * (dtl + 1), :]
                            .rearrange("u p t -> p u t"),
                            in_=att_sb[dtl][:])

                # staggered emission: scores for k-tile k+2 run while k's
                # probs finish on scalar/vector; softmax finalize for the
                # first q-half lands mid-instance, the second is deferred
                # into the next instance's stream.
                qk(0)
                if NKT8 > 1:
                    qk(1)
                if prev_final:
                    prev_final()
                if post_prev:
                    post_prev()
                for k in range(NKT8):
                    if k + 2 < NKT8:
                        qk(k + 2)
                    spv(k)
                    if k == stops[0]:
                        dve_chain(0)
                    if NQH > 1 and k == stops[0] + 1:
                        bcast_norm(0)
                if NQH == 1:
                    bcast_norm(0)
                    def final():
                        scatter()
                else:
                    dve_chain(1)

                    def final():
                        bcast_norm(1)
                        scatter()
                return final

            prev_final = None
            post_prev = None
            if attn_on:
                for hl in range(HPC):
                    for b in range(B):
                        prev_final = emit_instance(hl, b, prev_final, post_prev)
                        post_prev = None
                    post_prev = (lambda hl=hl: launch_a2a(hl))
                prev_final()
                post_prev()
                if 'proj' in phases:
                    for hl in range(HPC):
                        load_amt(hl)

            def am_sl(ki, mt, mm):
                hl, src, sub = kts[ki]
                base = (src * 2 + sub) * TS
                return am_all[hl][:, base + 128 * mt:base + 128 * mt + mm]

            PKB = 8  # k-tiles per w block
            NKB = (len(kts) + PKB - 1) // PKB
            NSP = 6 if NNT else 0  # nt's whose head-0 part runs early
            stash = {}

            def proj_part(nt, kbs):
                ps_f = [avps_pool.tile([min(128, TS), 512], dt.float32,
                                       tag=f"av{mt // 2}{mt % 2}",
                                       name=f"f{mt}_{nt}_{kbs[0]}")
                        for mt in range(NMT)]
                for kb in kbs:
                    kis = list(range(PKB * kb, min(PKB * (kb + 1), len(kts))))
                    wblk = ow_pool.tile([128, PKB * 512], bf16, tag="wr",
                                        name=f"wr_{nt}_{kb}")
                    # the first few weight blocks have no dependencies:
                    # stage them mid-QKV so the projection start never
                    # waits on its weight stream
                    early = nt == 0 or (nt == 1 and kb == kbs[0])
                    ctx = tc.high_priority() if early else None
                    if ctx:
                        ctx.__enter__()
                    with tc.tile_wait_until(0.5, enable=early):
                        nc.sync.dma_start(
                            out=wblk[:],
                            in_=wout_in.ap()[:, PKB * kb:PKB * (kb + 1),
                                             512 * nt:512 * (nt + 1)])
                    if ctx:
                        ctx.__exit__(None, None, None)
                    for mt in range(NMT):
                        mm = min(128, TS - 128 * mt)
                        for i, ki in enumerate(kis):
                            nc.tensor.matmul(
                                out=ps_f[mt][:],
                                lhsT=am_sl(ki, mt, mm),
                                rhs=wblk[:, 512 * i:512 * (i + 1)],
                                start=(ki == PKB * kbs[0]),
                                stop=(ki == PKB * (kbs[-1] + 1) - 1))
                return ps_f

            # head-0 half for the first NSP nt's (overlaps a2a of head 1)
            for nt in range(NSP):
                ps_f = proj_part(nt, list(range(NKB // 2)))
                for mt in range(NMT):
                    st = oa_pool.tile([min(128, TS), 512], bf16,
                                      tag=f"sth{nt}{mt}", name=f"sth{nt}{mt}")
                    nc.vector.tensor_copy(out=st[:], in_=ps_f[mt][:])
                    stash[(nt, mt)] = st
            for nt in range(NNT):
                if nt < NSP:
                    ps_f = proj_part(nt, list(range(NKB // 2, NKB)))
                else:
                    ps_f = proj_part(nt, list(range(NKB)))
                for mt in range(NMT):
                    mm = min(128, TS - 128 * mt)
                    fo = of_pool.tile([min(128, TS), 512], f32, tag="fo")
                    if nt < NSP:
                        nc.vector.tensor_add(fo[:], ps_f[mt][:],
                                             stash[(nt, mt)][:])
                    else:
                        nc.scalar.copy(out=fo[:], in_=ps_f[mt][:])
                    nc.scalar.dma_start(
                        out=out_f.ap()[128 * mt:128 * mt + mm,
                                       512 * nt:512 * (nt + 1)],
                        in_=fo[:])

    nc.compile()
    return nc


def get_nc(S, phases=('qkv', 'attn', 'proj')):
    key = (S, tuple(phases))
    if key not in _BUILD_CACHE:
        _BUILD_CACHE[key] = build(S, phases=phases)
    return _BUILD_CACHE[key]


def make_in_maps(position_ids, hidden_states, w_qkv, w_out):
    import ml_dtypes
    S = hidden_states.shape[1]
    TOK = B * S
    flat = np.asarray(hidden_states, dtype=np.float32).reshape(TOK, HID)
    hidT = flat.T.astype(ml_dtypes.bfloat16)          # [HID, TOK]
    hidT_t = np.ascontiguousarray(
        hidT.reshape(KT, 128, TOK).transpose(1, 0, 2))  # [128, KT, TOK]
    pos = np.asarray(position_ids).reshape(TOK).astype(np.float32)
    invf = (1.0 / (ROPE_BASE ** (np.arange(0, ROT, 2, dtype=np.float32) / ROT)))
    ang = invf[:, None] * pos[None, :]  # [RH, TOK]
    cos_t = np.cos(ang).astype(np.float32)
    sin_t = np.sin(ang).astype(np.float32)
    w_qkv = np.asarray(w_qkv, dtype=np.float32)
    w_out = np.asarray(w_out, dtype=np.float32)
    kts = [(hl, src, sub) for hl in range(HPC)
           for src in range(N_CORES) for sub in range(2)]
    wout_rows = np.stack([
        w_out[512 * src + 256 * hl + 128 * sub:
              512 * src + 256 * hl + 128 * sub + 128]
        for (hl, src, sub) in kts])                    # [32, 128, HID]
    wout_t = np.ascontiguousarray(
        wout_rows.transpose(1, 0, 2)).astype(ml_dtypes.bfloat16)  # [128, 32, HID]
    in_maps = []
    for c in range(N_CORES):
        c0 = HPC * D * c
        wq = np.concatenate([w_qkv[:, c0:c0 + HPC * D],
                             w_qkv[:, HID + c0:HID + c0 + HPC * D],
                             w_qkv[:, 2 * HID + c0:2 * HID + c0 + HPC * D]],
                            axis=1)                    # [HID, QKV_COLS]
        wq_t = np.ascontiguousarray(
            wq.reshape(KT, 128, QKV_COLS).transpose(1, 0, 2)
        ).astype(ml_dtypes.bfloat16)                   # [128, KT, QKV_COLS]
        in_maps.append({
            "cos_t": cos_t,
            "sin_t": sin_t,
            "hidT_t": hidT_t,
            "w_qkv_t": wq_t,
            "w_out_t": wout_t,
        })
    return in_maps


def kernel(position_ids, hidden_states, w_qkv, w_out):
    S = hidden_states.shape[1]
    nc = get_nc(S)
    in_maps = make_in_maps(position_ids, hidden_states, w_qkv, w_out)
    res = run_bass_kernel_spmd(nc, in_maps, list(range(N_CORES)))
    TOK = B * S
    out = np.concatenate([res.results[c]["out_f_0"] for c in range(N_CORES)], axis=0)
    return out.reshape(B, S, HID).astype(np.float32)
